# revision 4
# baseline (speedup 1.0000x reference)
"""Bilinear 2x upsample (16,3,512,512)->(16,3,1024,1024) on 8 trn2 NeuronCores.

Exact 2x bilinear: src = dst * 0.5, so
  out[2r, 2c]     = x[r, c]
  out[2r, 2c+1]   = 0.5*x[r, c]   + 0.5*x[r, c+1]   (clamped at c=511)
  out[2r+1, *]    = 0.5*row(2r,*) + 0.5*row(2r+2,*) (clamped at r=511)
All scale factors are powers of two, so the kernel reproduces the
reference bit-exactly (mul by 0.5 is exact; the adds round identically).

Sharding: pure data parallel, 2 images (= 6 512x512 planes) per core.

Per-core layout: each plane is loaded as T[128, 4, 512] with input row
r = 4p + b (partition p, free-dim block b). Horizontal interpolation
produces interleaved rows H[128, 4, 1024]; vertical averaging between
consecutive rows happens inside a partition (free-dim block shift) for
3/4 of the rows, and via a one-partition-shift SBUF->SBUF DMA of the
halved rows for the remaining block boundary (row 4p+3 pairs with row
4(p+1) which lives one partition down).
"""

import sys

if "/opt/trn_rl_repo" not in sys.path:
    sys.path.insert(0, "/opt/trn_rl_repo")

import numpy as np

N_CORES = 8
N, C, HI, WI = 16, 3, 512, 512
HO, WO = 1024, 1024
PLANES = (N // N_CORES) * C  # 6 planes per core
P = 128
B = HI // P  # 4 row-blocks per partition

_cached = {}


def _split_excess_waits(nc, max_waits=1):
    """Hoist excess sem waits into no-ops so each instruction carries <=max_waits.

    The walrus build in this container rejects instructions carrying more
    sync-wait commands than the ISA encoding slot count ("Too many sync wait
    commands", e.g. TPB_CTRL holds 1). Tile's scheduler attaches one wait per
    producer proc to a single instruction through an unchecked path. Waiting on
    a chain of same-engine no-ops immediately before the instruction is
    semantically identical (the engine stream is sequential), so move the
    excess waits there.
    """
    import concourse.mybir as mybir

    for f in nc.m.functions:
        for bb in f.blocks:
            insts = bb.instructions
            if not any(
                i.sync_info is not None and len(i.sync_info.on_wait) > max_waits
                for i in insts
            ):
                continue
            new = []
            for inst in insts:
                si = inst.sync_info
                if si is not None and len(si.on_wait) > max_waits:
                    waits = list(si.on_wait)
                    for w in waits[max_waits:]:
                        nop = mybir.InstNoOp(
                            name=nc.get_next_instruction_name(),
                            engine=inst.engine,
                            sync_info=mybir.SyncInfo(on_wait=[w], on_update=[]),
                            bass_nofuse=True,
                        )
                        nc.register_instruction(nop, overwrite=True)
                        new.append(nop)
                    inst.sync_info = mybir.SyncInfo(
                        on_wait=waits[:max_waits], on_update=list(si.on_update)
                    )
                new.append(inst)
            bb.instructions = new


def _build_module(reps=1, bufs=4):
    import concourse.bass as bass
    import concourse.mybir as mybir
    import concourse.tile as tile

    f32 = mybir.dt.float32
    bf16 = mybir.dt.bfloat16
    nc = bass.Bass()
    # x is the host-pre-gathered tile layout: [plane, partition, 5*512]
    # with x[pl, p, b*512 + w] = image[pl, min(4p+b, 511), w].
    x = nc.dram_tensor("x", [PLANES, P, (B + 1) * WI], f32, kind="ExternalInput")
    # Output is stored bf16 (rel err <= 2^-9, far under the 2e-2 gate) and
    # upcast to fp32 on the host: halves the dominant store traffic.
    out = nc.dram_tensor("out", [PLANES, HO, WO], bf16, kind="ExternalOutput")

    B5 = B + 1  # 4 owned row-blocks + 1 overlap block (row 4p+4)
    with tile.TileContext(nc) as tc:
        with tc.tile_pool(name="pool", bufs=bufs) as pool:
            for pl in [p for _ in range(reps) for p in range(PLANES)]:
                # ---- load t5[p, b, w] = x[pl, min(4p+b, 511), w], b=0..4.
                # The overlapped+clamped row layout is pre-gathered on the
                # host (see _prep), so this is a single [128, 10KB-contiguous]
                # DMA — the only load shape that runs at full HBM bandwidth
                # here (127-partition, strided, or single-row DMAs measured
                # 10-40x slower).
                t5 = pool.tile([P, B5, WI], f32)
                nc.sync.dma_start(t5[:], x[:][pl].rearrange("p (b w) -> p b w", b=B5))

                # All fp32 intermediates below reproduce the reference values
                # bit-exactly (mul by 0.5/2.0 is exact; add-then-halve rounds
                # identically to reference's halve-then-add). Each output
                # element takes exactly ONE extra bf16 rounding at the final
                # write, so |out - ref| <= |ref| * 2^-9 elementwise — tiny
                # absolute error even where averages cancel to ~0.
                nc.scalar.mul(t5[:], t5[:], 0.5)  # th = x/2, in place

                # fp32 odd-column h values: hfo[b, j] = h(row, 2j+1)
                hfo = pool.tile([P, B5, WI], f32)
                nc.vector.tensor_add(
                    hfo[:, :, 0 : WI - 1], t5[:, :, 0 : WI - 1], t5[:, :, 1:WI]
                )
                nc.scalar.mul(hfo[:, :, WI - 1 : WI], t5[:, :, WI - 1 : WI], 2.0)

                # Combined bf16 store tile: hv[p, b, 0, :] = out row 8p+2b
                # (horizontal pass), hv[p, b, 1, :] = out row 8p+2b+1
                # (vertical average) — one fully-contiguous 16KB/partition
                # store per plane.
                hv = pool.tile([P, B, 2, WO], bf16)
                nc.scalar.mul(hv[:, :, 0, 0:WO:2], t5[:, 0:B, :], 2.0)
                nc.gpsimd.tensor_copy(hv[:, :, 0, 1:WO:2], hfo[:, 0:B, :])

                # v even cols: th_r + th_{r+1} (== ref 0.5*t_r + 0.5*t_{r+1})
                nc.vector.tensor_add(
                    hv[:, :, 1, 0:WO:2], t5[:, 0:B, :], t5[:, 1:B5, :]
                )
                # v odd cols: 0.5 * (h_odd_r + h_odd_{r+1})
                vso = pool.tile([P, B, WI], f32)
                nc.vector.tensor_add(vso[:], hfo[:, 0:B, :], hfo[:, 1:B5, :])
                nc.scalar.mul(hv[:, :, 1, 1:WO:2], vso[:], 0.5)

                # ---- store: rows 8p..8p+7 of the plane, contiguous
                nc.sync.dma_start(
                    out[:][pl].rearrange("(p b e) w -> p b e w", b=B, e=2), hv[:]
                )

    _split_excess_waits(nc)
    nc.finalize()
    return nc


def _get_module():
    if "nc" not in _cached:
        _cached["nc"] = _build_module()
    return _cached["nc"]


_ROW_IDX = np.minimum(
    4 * np.arange(P)[:, None] + np.arange(B + 1)[None, :], HI - 1
)  # [128, 5] source row per (partition, block)


def _prep(planes):
    """[n_planes, 512, 512] image planes -> [n_planes, 128, 2560] tile layout."""
    g = planes[:, _ROW_IDX, :]  # [n, 128, 5, 512]
    return np.ascontiguousarray(g.reshape(planes.shape[0], P, (B + 1) * WI))


def kernel(x, target_height=1024, target_width=1024):
    from concourse.bass_utils import run_bass_kernel_spmd

    assert int(target_height) == HO and int(target_width) == WO
    x = np.asarray(x, dtype=np.float32)
    assert x.shape == (N, C, HI, WI)
    xg = _prep(x.reshape(N * C, HI, WI))  # [48, 128, 2560]

    nc = _get_module()
    per_core = N // N_CORES
    in_maps = [
        {"x": xg[i * PLANES : (i + 1) * PLANES]} for i in range(N_CORES)
    ]
    res = run_bass_kernel_spmd(nc, in_maps, core_ids=list(range(N_CORES)))
    out = np.concatenate(
        [
            np.asarray(r["out"]).astype(np.float32).reshape(per_core, C, HO, WO)
            for r in res.results
        ],
        axis=0,
    )
    return out



# revision 6
# speedup vs baseline: 1.5999x; 1.5999x over previous
"""Bilinear 2x upsample (16,3,512,512)->(16,3,1024,1024) on 8 trn2 NeuronCores.

Exact 2x bilinear: src = dst * 0.5, so
  out[2r, 2c]     = x[r, c]
  out[2r, 2c+1]   = 0.5*x[r, c]   + 0.5*x[r, c+1]   (clamped at c=511)
  out[2r+1, *]    = 0.5*row(2r,*) + 0.5*row(2r+2,*) (clamped at r=511)
All scale factors are powers of two, so the kernel reproduces the
reference bit-exactly (mul by 0.5 is exact; the adds round identically).

Sharding: pure data parallel, 2 images (= 6 512x512 planes) per core.

Per-core layout: each plane is loaded as T[128, 4, 512] with input row
r = 4p + b (partition p, free-dim block b). Horizontal interpolation
produces interleaved rows H[128, 4, 1024]; vertical averaging between
consecutive rows happens inside a partition (free-dim block shift) for
3/4 of the rows, and via a one-partition-shift SBUF->SBUF DMA of the
halved rows for the remaining block boundary (row 4p+3 pairs with row
4(p+1) which lives one partition down).
"""

import sys

if "/opt/trn_rl_repo" not in sys.path:
    sys.path.insert(0, "/opt/trn_rl_repo")

import numpy as np

N_CORES = 8
N, C, HI, WI = 16, 3, 512, 512
HO, WO = 1024, 1024
PLANES = (N // N_CORES) * C  # 6 planes per core
P = 128
B = HI // P  # 4 row-blocks per partition

_cached = {}


def _split_excess_waits(nc, max_waits=1):
    """Hoist excess sem waits into no-ops so each instruction carries <=max_waits.

    The walrus build in this container rejects instructions carrying more
    sync-wait commands than the ISA encoding slot count ("Too many sync wait
    commands", e.g. TPB_CTRL holds 1). Tile's scheduler attaches one wait per
    producer proc to a single instruction through an unchecked path. Waiting on
    a chain of same-engine no-ops immediately before the instruction is
    semantically identical (the engine stream is sequential), so move the
    excess waits there.
    """
    import concourse.mybir as mybir

    for f in nc.m.functions:
        for bb in f.blocks:
            insts = bb.instructions
            if not any(
                i.sync_info is not None and len(i.sync_info.on_wait) > max_waits
                for i in insts
            ):
                continue
            new = []
            for inst in insts:
                si = inst.sync_info
                if si is not None and len(si.on_wait) > max_waits:
                    waits = list(si.on_wait)
                    for w in waits[max_waits:]:
                        nop = mybir.InstNoOp(
                            name=nc.get_next_instruction_name(),
                            engine=inst.engine,
                            sync_info=mybir.SyncInfo(on_wait=[w], on_update=[]),
                            bass_nofuse=True,
                        )
                        nc.register_instruction(nop, overwrite=True)
                        new.append(nop)
                    inst.sync_info = mybir.SyncInfo(
                        on_wait=waits[:max_waits], on_update=list(si.on_update)
                    )
                new.append(inst)
            bb.instructions = new


def _build_module(reps=1, bufs=4):
    import concourse.bass as bass
    import concourse.mybir as mybir
    import concourse.tile as tile

    f32 = mybir.dt.float32
    bf16 = mybir.dt.bfloat16
    nc = bass.Bass()
    # x is the host-pre-gathered tile layout: [plane, partition, 5*512]
    # with x[pl, p, b*512 + w] = image[pl, min(4p+b, 511), w].
    x = nc.dram_tensor("x", [PLANES, P, (B + 1) * WI], f32, kind="ExternalInput")
    # Output is stored bf16 (rel err <= 2^-9, far under the 2e-2 gate) and
    # upcast to fp32 on the host: halves the dominant store traffic.
    out = nc.dram_tensor("out", [PLANES, HO, WO], bf16, kind="ExternalOutput")

    B5 = B + 1  # 4 owned row-blocks + 1 overlap block (row 4p+4)
    with tile.TileContext(nc) as tc:
        with tc.tile_pool(name="pool", bufs=bufs) as pool:
            for pl in [p for _ in range(reps) for p in range(PLANES)]:
                # ---- load t5[p, b, w] = x[pl, min(4p+b, 511), w], b=0..4.
                # The overlapped+clamped row layout is pre-gathered on the
                # host (see _prep), so this is a single [128, 10KB-contiguous]
                # DMA — the only load shape that runs at full HBM bandwidth
                # here (127-partition, strided, or single-row DMAs measured
                # 10-40x slower).
                t5 = pool.tile([P, B5, WI], f32)
                nc.sync.dma_start(t5[:], x[:][pl].rearrange("p (b w) -> p b w", b=B5))

                # All fp32 intermediates below are bit-exact multiples of the
                # reference values (power-of-two scaling commutes with fp32
                # rounding; each interpolation sum rounds exactly once, same
                # as the reference). Each stored element then takes exactly
                # ONE bf16 rounding, so |out - ref| <= |ref| * 2^-9 even
                # where averages cancel to ~0. v_even is stored at half
                # scale (DVE adds can't scale their output) and fixed up by
                # an exact x2 on the host.
                nc.scalar.mul(t5[:], t5[:], 0.25)  # tq = x/4, in place

                # hq[b, j] = h(row, 2j+1) / 2  (fp32; = tq_j + tq_{j+1})
                hq = pool.tile([P, B5, WI], f32)
                nc.vector.tensor_add(
                    hq[:, :, 0 : WI - 1], t5[:, :, 0 : WI - 1], t5[:, :, 1:WI]
                )
                nc.scalar.mul(hq[:, :, WI - 1 : WI], t5[:, :, WI - 1 : WI], 2.0)

                # Combined bf16 store tile: hv[p, b, 0, :] = out row 8p+2b
                # (horizontal pass), hv[p, b, 1, :] = out row 8p+2b+1
                # (vertical average) — one fully-contiguous 16KB/partition
                # store per plane.
                hv = pool.tile([P, B, 2, WO], bf16)
                nc.scalar.mul(hv[:, :, 0, 0:WO:2], t5[:, 0:B, :], 4.0)
                nc.scalar.mul(hv[:, :, 0, 1:WO:2], hq[:, 0:B, :], 2.0)
                # v even cols at HALF scale: tq_r + tq_{r+1} (host multiplies
                # by 2); v odd cols exact: hq_r + hq_{r+1} == 0.5*(ho_r+ho_{r+1})
                nc.vector.tensor_add(
                    hv[:, :, 1, 0:WO:2], t5[:, 0:B, :], t5[:, 1:B5, :]
                )
                nc.vector.tensor_add(
                    hv[:, :, 1, 1:WO:2], hq[:, 0:B, :], hq[:, 1:B5, :]
                )

                # ---- store: rows 8p..8p+7 of the plane, contiguous
                nc.sync.dma_start(
                    out[:][pl].rearrange("(p b e) w -> p b e w", b=B, e=2), hv[:]
                )

    _split_excess_waits(nc)
    nc.finalize()
    return nc


def _get_module():
    if "nc" not in _cached:
        _cached["nc"] = _build_module()
    return _cached["nc"]


_ROW_IDX = np.minimum(
    4 * np.arange(P)[:, None] + np.arange(B + 1)[None, :], HI - 1
)  # [128, 5] source row per (partition, block)


def _prep(planes):
    """[n_planes, 512, 512] image planes -> [n_planes, 128, 2560] tile layout."""
    g = planes[:, _ROW_IDX, :]  # [n, 128, 5, 512]
    return np.ascontiguousarray(g.reshape(planes.shape[0], P, (B + 1) * WI))


def kernel(x, target_height=1024, target_width=1024):
    from concourse.bass_utils import run_bass_kernel_spmd

    assert int(target_height) == HO and int(target_width) == WO
    x = np.asarray(x, dtype=np.float32)
    assert x.shape == (N, C, HI, WI)
    xg = _prep(x.reshape(N * C, HI, WI))  # [48, 128, 2560]

    nc = _get_module()
    per_core = N // N_CORES
    in_maps = [
        {"x": xg[i * PLANES : (i + 1) * PLANES]} for i in range(N_CORES)
    ]
    res = run_bass_kernel_spmd(nc, in_maps, core_ids=list(range(N_CORES)))
    out = np.concatenate(
        [
            np.asarray(r["out"]).astype(np.float32).reshape(per_core, C, HO, WO)
            for r in res.results
        ],
        axis=0,
    )
    # v_even elements (odd rows, even cols) were stored at half scale.
    out[:, :, 1::2, 0::2] *= 2.0
    return out



# revision 8
# speedup vs baseline: 1.6592x; 1.0371x over previous
"""Bilinear 2x upsample (16,3,512,512)->(16,3,1024,1024) on 8 trn2 NeuronCores.

Exact 2x bilinear: src = dst * 0.5, so
  out[2r, 2c]     = x[r, c]
  out[2r, 2c+1]   = 0.5*x[r, c]   + 0.5*x[r, c+1]   (clamped at c=511)
  out[2r+1, *]    = 0.5*row(2r,*) + 0.5*row(2r+2,*) (clamped at r=511)
All scale factors are powers of two, so the kernel reproduces the
reference bit-exactly (mul by 0.5 is exact; the adds round identically).

Sharding: pure data parallel, 2 images (= 6 512x512 planes) per core.

Per-core layout: each plane is loaded as T[128, 4, 512] with input row
r = 4p + b (partition p, free-dim block b). Horizontal interpolation
produces interleaved rows H[128, 4, 1024]; vertical averaging between
consecutive rows happens inside a partition (free-dim block shift) for
3/4 of the rows, and via a one-partition-shift SBUF->SBUF DMA of the
halved rows for the remaining block boundary (row 4p+3 pairs with row
4(p+1) which lives one partition down).
"""

import sys

if "/opt/trn_rl_repo" not in sys.path:
    sys.path.insert(0, "/opt/trn_rl_repo")

import numpy as np

N_CORES = 8
N, C, HI, WI = 16, 3, 512, 512
HO, WO = 1024, 1024
PLANES = (N // N_CORES) * C  # 6 planes per core
P = 128
B = HI // P  # 4 row-blocks per partition

_cached = {}


def _split_excess_waits(nc, max_waits=1):
    """Hoist excess sem waits into no-ops so each instruction carries <=max_waits.

    The walrus build in this container rejects instructions carrying more
    sync-wait commands than the ISA encoding slot count ("Too many sync wait
    commands", e.g. TPB_CTRL holds 1). Tile's scheduler attaches one wait per
    producer proc to a single instruction through an unchecked path. Waiting on
    a chain of same-engine no-ops immediately before the instruction is
    semantically identical (the engine stream is sequential), so move the
    excess waits there.
    """
    import concourse.mybir as mybir

    for f in nc.m.functions:
        for bb in f.blocks:
            insts = bb.instructions
            if not any(
                i.sync_info is not None and len(i.sync_info.on_wait) > max_waits
                for i in insts
            ):
                continue
            new = []
            for inst in insts:
                si = inst.sync_info
                if si is not None and len(si.on_wait) > max_waits:
                    waits = list(si.on_wait)
                    for w in waits[max_waits:]:
                        nop = mybir.InstNoOp(
                            name=nc.get_next_instruction_name(),
                            engine=inst.engine,
                            sync_info=mybir.SyncInfo(on_wait=[w], on_update=[]),
                            bass_nofuse=True,
                        )
                        nc.register_instruction(nop, overwrite=True)
                        new.append(nop)
                    inst.sync_info = mybir.SyncInfo(
                        on_wait=waits[:max_waits], on_update=list(si.on_update)
                    )
                new.append(inst)
            bb.instructions = new


def _build_module(reps=1, bufs=4):
    import concourse.bass as bass
    import concourse.mybir as mybir
    import concourse.tile as tile

    f32 = mybir.dt.float32
    bf16 = mybir.dt.bfloat16
    nc = bass.Bass()
    # x is the host-pre-gathered tile layout: [plane, partition, 5*512]
    # with x[pl, p, b*512 + w] = image[pl, min(4p+b, 511), w].
    x = nc.dram_tensor("x", [PLANES, P, (B + 1) * WI], f32, kind="ExternalInput")
    # Output is stored bf16 (rel err <= 2^-9, far under the 2e-2 gate) and
    # upcast to fp32 on the host: halves the dominant store traffic.
    out = nc.dram_tensor("out", [PLANES, HO, WO], bf16, kind="ExternalOutput")

    B5 = B + 1  # 4 owned row-blocks + 1 overlap block (row 4p+4)
    with tile.TileContext(nc) as tc:
        with (
            tc.tile_pool(name="tpool", bufs=PLANES) as tpool,
            tc.tile_pool(name="pool", bufs=bufs) as pool,
        ):
            for pl in [p for _ in range(reps) for p in range(PLANES)]:
                # ---- load t5[p, b, w] = x[pl, min(4p+b, 511), w], b=0..4.
                # The overlapped+clamped row layout is pre-gathered on the
                # host (see _prep), so this is a single [128, 10KB-contiguous]
                # DMA — the only load shape that runs at full HBM bandwidth
                # here (127-partition, strided, or single-row DMAs measured
                # 10-40x slower).
                t5 = tpool.tile([P, B5, WI], f32)
                nc.sync.dma_start(t5[:], x[:][pl].rearrange("p (b w) -> p b w", b=B5))

                # All fp32 intermediates below are bit-exact multiples of the
                # reference values (power-of-two scaling commutes with fp32
                # rounding; each interpolation sum rounds exactly once, same
                # as the reference). Each stored element then takes exactly
                # ONE bf16 rounding, so |out - ref| <= |ref| * 2^-9 even
                # where averages cancel to ~0. v_even is stored at half
                # scale (DVE adds can't scale their output) and fixed up by
                # an exact x2 on the host.
                nc.scalar.mul(t5[:], t5[:], 0.25)  # tq = x/4, in place

                # hq[b, j] = h(row, 2j+1) / 2  (fp32; = tq_j + tq_{j+1})
                hq = pool.tile([P, B5, WI], f32)
                nc.vector.tensor_add(
                    hq[:, :, 0 : WI - 1], t5[:, :, 0 : WI - 1], t5[:, :, 1:WI]
                )
                nc.scalar.mul(hq[:, :, WI - 1 : WI], t5[:, :, WI - 1 : WI], 2.0)

                # Combined bf16 store tile: hv[p, b, 0, :] = out row 8p+2b
                # (horizontal pass), hv[p, b, 1, :] = out row 8p+2b+1
                # (vertical average) — one fully-contiguous 16KB/partition
                # store per plane.
                hv = pool.tile([P, B, 2, WO], bf16)
                nc.scalar.mul(hv[:, :, 0, 0:WO:2], t5[:, 0:B, :], 4.0)
                nc.scalar.mul(hv[:, :, 0, 1:WO:2], hq[:, 0:B, :], 2.0)
                # v even cols at HALF scale: tq_r + tq_{r+1} (host multiplies
                # by 2); v odd cols exact: hq_r + hq_{r+1} == 0.5*(ho_r+ho_{r+1})
                nc.vector.tensor_add(
                    hv[:, :, 1, 0:WO:2], t5[:, 0:B, :], t5[:, 1:B5, :]
                )
                nc.vector.tensor_add(
                    hv[:, :, 1, 1:WO:2], hq[:, 0:B, :], hq[:, 1:B5, :]
                )

                # ---- store: rows 8p..8p+7 of the plane, contiguous;
                # two half-plane stores so the first can start while the
                # second half is still being computed.
                dst = out[:][pl].rearrange("(p b e) w -> p b e w", b=B, e=2)
                nc.sync.dma_start(dst[:, 0 : B // 2], hv[:, 0 : B // 2])
                nc.sync.dma_start(dst[:, B // 2 : B], hv[:, B // 2 : B])

    _split_excess_waits(nc)
    nc.finalize()
    return nc


def _get_module():
    if "nc" not in _cached:
        _cached["nc"] = _build_module()
    return _cached["nc"]


_ROW_IDX = np.minimum(
    4 * np.arange(P)[:, None] + np.arange(B + 1)[None, :], HI - 1
)  # [128, 5] source row per (partition, block)


def _prep(planes):
    """[n_planes, 512, 512] image planes -> [n_planes, 128, 2560] tile layout."""
    g = planes[:, _ROW_IDX, :]  # [n, 128, 5, 512]
    return np.ascontiguousarray(g.reshape(planes.shape[0], P, (B + 1) * WI))


def kernel(x, target_height=1024, target_width=1024):
    from concourse.bass_utils import run_bass_kernel_spmd

    assert int(target_height) == HO and int(target_width) == WO
    x = np.asarray(x, dtype=np.float32)
    assert x.shape == (N, C, HI, WI)
    xg = _prep(x.reshape(N * C, HI, WI))  # [48, 128, 2560]

    nc = _get_module()
    per_core = N // N_CORES
    in_maps = [
        {"x": xg[i * PLANES : (i + 1) * PLANES]} for i in range(N_CORES)
    ]
    res = run_bass_kernel_spmd(nc, in_maps, core_ids=list(range(N_CORES)))
    out = np.concatenate(
        [
            np.asarray(r["out"]).astype(np.float32).reshape(per_core, C, HO, WO)
            for r in res.results
        ],
        axis=0,
    )
    # v_even elements (odd rows, even cols) were stored at half scale.
    out[:, :, 1::2, 0::2] *= 2.0
    return out



# revision 9
# speedup vs baseline: 1.8954x; 1.1423x over previous
"""Bilinear 2x upsample (16,3,512,512)->(16,3,1024,1024) on 8 trn2 NeuronCores.

Exact 2x bilinear: src = dst * 0.5, so
  out[2r, 2c]     = x[r, c]
  out[2r, 2c+1]   = 0.5*x[r, c]   + 0.5*x[r, c+1]   (clamped at c=511)
  out[2r+1, *]    = 0.5*row(2r,*) + 0.5*row(2r+2,*) (clamped at r=511)
All scale factors are powers of two, so the kernel reproduces the
reference bit-exactly (mul by 0.5 is exact; the adds round identically).

Sharding: pure data parallel, 2 images (= 6 512x512 planes) per core.

Per-core layout: each plane is loaded as T[128, 4, 512] with input row
r = 4p + b (partition p, free-dim block b). Horizontal interpolation
produces interleaved rows H[128, 4, 1024]; vertical averaging between
consecutive rows happens inside a partition (free-dim block shift) for
3/4 of the rows, and via a one-partition-shift SBUF->SBUF DMA of the
halved rows for the remaining block boundary (row 4p+3 pairs with row
4(p+1) which lives one partition down).
"""

import sys

if "/opt/trn_rl_repo" not in sys.path:
    sys.path.insert(0, "/opt/trn_rl_repo")

import numpy as np

N_CORES = 8
N, C, HI, WI = 16, 3, 512, 512
HO, WO = 1024, 1024
PLANES = (N // N_CORES) * C  # 6 planes per core
P = 128
B = HI // P  # 4 row-blocks per partition

_cached = {}


def _split_excess_waits(nc, max_waits=1):
    """Hoist excess sem waits into no-ops so each instruction carries <=max_waits.

    The walrus build in this container rejects instructions carrying more
    sync-wait commands than the ISA encoding slot count ("Too many sync wait
    commands", e.g. TPB_CTRL holds 1). Tile's scheduler attaches one wait per
    producer proc to a single instruction through an unchecked path. Waiting on
    a chain of same-engine no-ops immediately before the instruction is
    semantically identical (the engine stream is sequential), so move the
    excess waits there.
    """
    import concourse.mybir as mybir

    for f in nc.m.functions:
        for bb in f.blocks:
            insts = bb.instructions
            if not any(
                i.sync_info is not None and len(i.sync_info.on_wait) > max_waits
                for i in insts
            ):
                continue
            new = []
            for inst in insts:
                si = inst.sync_info
                if si is not None and len(si.on_wait) > max_waits:
                    waits = list(si.on_wait)
                    for w in waits[max_waits:]:
                        nop = mybir.InstNoOp(
                            name=nc.get_next_instruction_name(),
                            engine=inst.engine,
                            sync_info=mybir.SyncInfo(on_wait=[w], on_update=[]),
                            bass_nofuse=True,
                        )
                        nc.register_instruction(nop, overwrite=True)
                        new.append(nop)
                    inst.sync_info = mybir.SyncInfo(
                        on_wait=waits[:max_waits], on_update=list(si.on_update)
                    )
                new.append(inst)
            bb.instructions = new


def _build_module(reps=1, bufs=4):
    import concourse.bass as bass
    import concourse.mybir as mybir
    import concourse.tile as tile

    f32 = mybir.dt.float32
    bf16 = mybir.dt.bfloat16
    nc = bass.Bass()
    # x is the host-pre-gathered tile layout: [plane, partition, 5*512]
    # with x[pl, p, b*512 + w] = image[pl, min(4p+b, 511), w].
    x = nc.dram_tensor("x", [PLANES, P, (B + 1) * WI], f32, kind="ExternalInput")
    # Output is stored bf16 (rel err <= 2^-9, far under the 2e-2 gate) and
    # upcast to fp32 on the host: halves the dominant store traffic.
    out = nc.dram_tensor("out", [PLANES, HO, WO], bf16, kind="ExternalOutput")

    B5 = B + 1  # 4 owned row-blocks + 1 overlap block (row 4p+4)
    with tile.TileContext(nc) as tc:
        with (
            tc.tile_pool(name="tpool", bufs=PLANES) as tpool,
            tc.tile_pool(name="pool", bufs=bufs) as pool,
        ):
            # ---- loads: t5[p, b, w] = x[pl, min(4p+b, 511), w], b=0..4.
            # The overlapped+clamped row layout is pre-gathered on the host
            # (see _prep), so each load is a single [128, 10KB-contiguous]
            # DMA — the only load shape that runs at full HBM bandwidth here
            # (127-partition, strided, or single-row DMAs measured 10-40x
            # slower). All PLANES loads are issued up front from the Scalar
            # engine's HWDGE with no waits, so they queue back-to-back at
            # t=0 and never sit behind a store's compute-wait in the Sync
            # FIFO (head-of-line blocking).
            t5s = []
            for pl in range(PLANES):
                t5 = tpool.tile([P, B5, WI], f32)
                nc.scalar.dma_start(
                    t5[:], x[:][pl].rearrange("p (b w) -> p b w", b=B5)
                )
                t5s.append(t5)

            for pl in [p for _ in range(reps) for p in range(PLANES)]:
                t5 = t5s[pl]
                # All fp32 intermediates below are bit-exact multiples of the
                # reference values (power-of-two scaling commutes with fp32
                # rounding; each interpolation sum rounds exactly once, same
                # as the reference). Each stored element then takes exactly
                # ONE bf16 rounding, so |out - ref| <= |ref| * 2^-9 even
                # where averages cancel to ~0. v_even is stored at half
                # scale (DVE adds can't scale their output) and fixed up by
                # an exact x2 on the host.
                nc.scalar.mul(t5[:], t5[:], 0.25)  # tq = x/4, in place

                # hq[b, j] = h(row, 2j+1) / 2  (fp32; = tq_j + tq_{j+1})
                hq = pool.tile([P, B5, WI], f32)
                nc.vector.tensor_add(
                    hq[:, :, 0 : WI - 1], t5[:, :, 0 : WI - 1], t5[:, :, 1:WI]
                )
                nc.scalar.mul(hq[:, :, WI - 1 : WI], t5[:, :, WI - 1 : WI], 2.0)

                # Combined bf16 store tile: hv[p, b, 0, :] = out row 8p+2b
                # (horizontal pass), hv[p, b, 1, :] = out row 8p+2b+1
                # (vertical average) — one fully-contiguous 16KB/partition
                # store per plane.
                hv = pool.tile([P, B, 2, WO], bf16)
                nc.scalar.mul(hv[:, :, 0, 0:WO:2], t5[:, 0:B, :], 4.0)
                nc.scalar.mul(hv[:, :, 0, 1:WO:2], hq[:, 0:B, :], 2.0)
                # v even cols at HALF scale: tq_r + tq_{r+1} (host multiplies
                # by 2); v odd cols exact: hq_r + hq_{r+1} == 0.5*(ho_r+ho_{r+1})
                nc.vector.tensor_add(
                    hv[:, :, 1, 0:WO:2], t5[:, 0:B, :], t5[:, 1:B5, :]
                )
                nc.vector.tensor_add(
                    hv[:, :, 1, 1:WO:2], hq[:, 0:B, :], hq[:, 1:B5, :]
                )

                # ---- store: rows 8p..8p+7 of the plane, contiguous;
                # two half-plane stores so the first can start while the
                # second half is still being computed.
                dst = out[:][pl].rearrange("(p b e) w -> p b e w", b=B, e=2)
                nc.sync.dma_start(dst[:, 0 : B // 2], hv[:, 0 : B // 2])
                nc.sync.dma_start(dst[:, B // 2 : B], hv[:, B // 2 : B])

    _split_excess_waits(nc)
    nc.finalize()
    return nc


def _get_module():
    if "nc" not in _cached:
        _cached["nc"] = _build_module()
    return _cached["nc"]


_ROW_IDX = np.minimum(
    4 * np.arange(P)[:, None] + np.arange(B + 1)[None, :], HI - 1
)  # [128, 5] source row per (partition, block)


def _prep(planes):
    """[n_planes, 512, 512] image planes -> [n_planes, 128, 2560] tile layout."""
    g = planes[:, _ROW_IDX, :]  # [n, 128, 5, 512]
    return np.ascontiguousarray(g.reshape(planes.shape[0], P, (B + 1) * WI))


def kernel(x, target_height=1024, target_width=1024):
    from concourse.bass_utils import run_bass_kernel_spmd

    assert int(target_height) == HO and int(target_width) == WO
    x = np.asarray(x, dtype=np.float32)
    assert x.shape == (N, C, HI, WI)
    xg = _prep(x.reshape(N * C, HI, WI))  # [48, 128, 2560]

    nc = _get_module()
    per_core = N // N_CORES
    in_maps = [
        {"x": xg[i * PLANES : (i + 1) * PLANES]} for i in range(N_CORES)
    ]
    res = run_bass_kernel_spmd(nc, in_maps, core_ids=list(range(N_CORES)))
    out = np.concatenate(
        [
            np.asarray(r["out"]).astype(np.float32).reshape(per_core, C, HO, WO)
            for r in res.results
        ],
        axis=0,
    )
    # v_even elements (odd rows, even cols) were stored at half scale.
    out[:, :, 1::2, 0::2] *= 2.0
    return out



# revision 10
# speedup vs baseline: 1.9637x; 1.0360x over previous
"""Bilinear 2x upsample (16,3,512,512)->(16,3,1024,1024) on 8 trn2 NeuronCores.

Exact 2x bilinear: src = dst * 0.5, so
  out[2r, 2c]     = x[r, c]
  out[2r, 2c+1]   = 0.5*x[r, c]   + 0.5*x[r, c+1]   (clamped at c=511)
  out[2r+1, *]    = 0.5*row(2r,*) + 0.5*row(2r+2,*) (clamped at r=511)
All scale factors are powers of two, so the kernel reproduces the
reference bit-exactly (mul by 0.5 is exact; the adds round identically).

Sharding: pure data parallel, 2 images (= 6 512x512 planes) per core.

Per-core layout: each plane is loaded as T[128, 4, 512] with input row
r = 4p + b (partition p, free-dim block b). Horizontal interpolation
produces interleaved rows H[128, 4, 1024]; vertical averaging between
consecutive rows happens inside a partition (free-dim block shift) for
3/4 of the rows, and via a one-partition-shift SBUF->SBUF DMA of the
halved rows for the remaining block boundary (row 4p+3 pairs with row
4(p+1) which lives one partition down).
"""

import sys

if "/opt/trn_rl_repo" not in sys.path:
    sys.path.insert(0, "/opt/trn_rl_repo")

import numpy as np

N_CORES = 8
N, C, HI, WI = 16, 3, 512, 512
HO, WO = 1024, 1024
PLANES = (N // N_CORES) * C  # 6 planes per core
P = 128
B = HI // P  # 4 row-blocks per partition

_cached = {}


def _split_excess_waits(nc, max_waits=1):
    """Hoist excess sem waits into no-ops so each instruction carries <=max_waits.

    The walrus build in this container rejects instructions carrying more
    sync-wait commands than the ISA encoding slot count ("Too many sync wait
    commands", e.g. TPB_CTRL holds 1). Tile's scheduler attaches one wait per
    producer proc to a single instruction through an unchecked path. Waiting on
    a chain of same-engine no-ops immediately before the instruction is
    semantically identical (the engine stream is sequential), so move the
    excess waits there.
    """
    import concourse.mybir as mybir

    for f in nc.m.functions:
        for bb in f.blocks:
            insts = bb.instructions
            if not any(
                i.sync_info is not None and len(i.sync_info.on_wait) > max_waits
                for i in insts
            ):
                continue
            new = []
            for inst in insts:
                si = inst.sync_info
                if si is not None and len(si.on_wait) > max_waits:
                    waits = list(si.on_wait)
                    for w in waits[max_waits:]:
                        nop = mybir.InstNoOp(
                            name=nc.get_next_instruction_name(),
                            engine=inst.engine,
                            sync_info=mybir.SyncInfo(on_wait=[w], on_update=[]),
                            bass_nofuse=True,
                        )
                        nc.register_instruction(nop, overwrite=True)
                        new.append(nop)
                    inst.sync_info = mybir.SyncInfo(
                        on_wait=waits[:max_waits], on_update=list(si.on_update)
                    )
                new.append(inst)
            bb.instructions = new


def _build_module(reps=1, bufs=4):
    import concourse.bass as bass
    import concourse.mybir as mybir
    import concourse.tile as tile

    f32 = mybir.dt.float32
    bf16 = mybir.dt.bfloat16
    nc = bass.Bass()
    # x is the host-pre-gathered tile layout: [plane, partition, 5*512]
    # with x[pl, p, b*512 + w] = image[pl, min(4p+b, 511), w].
    x = nc.dram_tensor("x", [PLANES, P, (B + 1) * WI], f32, kind="ExternalInput")
    # Output is stored bf16 (rel err <= 2^-9, far under the 2e-2 gate) and
    # upcast to fp32 on the host: halves the dominant store traffic.
    out = nc.dram_tensor("out", [PLANES, HO, WO], bf16, kind="ExternalOutput")

    B5 = B + 1  # 4 owned row-blocks + 1 overlap block (row 4p+4)
    with tile.TileContext(nc) as tc:
        with (
            tc.tile_pool(name="tpool", bufs=PLANES) as tpool,
            tc.tile_pool(name="pool", bufs=bufs) as pool,
        ):
            # ---- loads: t5[p, b, w] = x[pl, min(4p+b, 511), w], b=0..4.
            # The overlapped+clamped row layout is pre-gathered on the host
            # (see _prep), so each load is a single [128, 10KB-contiguous]
            # DMA — the only load shape that runs at full HBM bandwidth here
            # (127-partition, strided, or single-row DMAs measured 10-40x
            # slower). All PLANES loads are issued up front from the Scalar
            # engine's HWDGE with no waits, so they queue back-to-back at
            # t=0 and never sit behind a store's compute-wait in the Sync
            # FIFO (head-of-line blocking).
            t5s = []
            for pl in range(PLANES):
                t5 = tpool.tile([P, B5, WI], f32)
                nc.sync.dma_start(
                    t5[:], x[:][pl].rearrange("p (b w) -> p b w", b=B5)
                )
                t5s.append(t5)

            for pl in [p for _ in range(reps) for p in range(PLANES)]:
                t5 = t5s[pl]
                # All fp32 intermediates below are bit-exact power-of-two
                # multiples of the reference values (power-of-two scaling
                # commutes with fp32 rounding; each interpolation sum rounds
                # exactly once, same as the reference). Each stored element
                # then takes exactly ONE bf16 rounding, so
                # |out - ref| <= |ref| * 2^-9 even where averages cancel to
                # ~0. v rows are stored at 2x/4x scale (DVE adds can't scale
                # their output) and fixed up by exact power-of-two divides on
                # the host.

                # hso[b, j] = 2 * h(row, 2j+1)  (fp32; = t_j + t_{j+1})
                hso = pool.tile([P, B5, WI], f32)
                nc.vector.tensor_add(
                    hso[:, :, 0 : WI - 1], t5[:, :, 0 : WI - 1], t5[:, :, 1:WI]
                )
                nc.scalar.mul(hso[:, :, WI - 1 : WI], t5[:, :, WI - 1 : WI], 2.0)

                # Combined bf16 store tile: hv[p, b, 0, :] = out row 8p+2b
                # (horizontal pass), hv[p, b, 1, :] = out row 8p+2b+1
                # (vertical average, scaled) — one contiguous 16KB/partition
                # store per plane.
                hv = pool.tile([P, B, 2, WO], bf16)
                nc.scalar.mul(hv[:, :, 0, 0:WO:2], t5[:, 0:B, :], 1.0)
                nc.scalar.mul(hv[:, :, 0, 1:WO:2], hso[:, 0:B, :], 0.5)
                # v even cols at 2x scale: t_r + t_{r+1}; v odd cols at 4x
                # scale: hso_r + hso_{r+1} == 4 * v_odd_ref
                nc.vector.tensor_add(
                    hv[:, :, 1, 0:WO:2], t5[:, 0:B, :], t5[:, 1:B5, :]
                )
                nc.vector.tensor_add(
                    hv[:, :, 1, 1:WO:2], hso[:, 0:B, :], hso[:, 1:B5, :]
                )

                # ---- store: rows 8p..8p+7 of the plane, contiguous;
                # two half-plane stores so the first can start while the
                # second half is still being computed.
                dst = out[:][pl].rearrange("(p b e) w -> p b e w", b=B, e=2)
                nc.sync.dma_start(dst[:, 0 : B // 2], hv[:, 0 : B // 2])
                nc.sync.dma_start(dst[:, B // 2 : B], hv[:, B // 2 : B])

    _split_excess_waits(nc)
    nc.finalize()
    return nc


def _get_module():
    if "nc" not in _cached:
        _cached["nc"] = _build_module()
    return _cached["nc"]


_ROW_IDX = np.minimum(
    4 * np.arange(P)[:, None] + np.arange(B + 1)[None, :], HI - 1
)  # [128, 5] source row per (partition, block)


def _prep(planes):
    """[n_planes, 512, 512] image planes -> [n_planes, 128, 2560] tile layout."""
    g = planes[:, _ROW_IDX, :]  # [n, 128, 5, 512]
    return np.ascontiguousarray(g.reshape(planes.shape[0], P, (B + 1) * WI))


def kernel(x, target_height=1024, target_width=1024):
    from concourse.bass_utils import run_bass_kernel_spmd

    assert int(target_height) == HO and int(target_width) == WO
    x = np.asarray(x, dtype=np.float32)
    assert x.shape == (N, C, HI, WI)
    xg = _prep(x.reshape(N * C, HI, WI))  # [48, 128, 2560]

    nc = _get_module()
    per_core = N // N_CORES
    in_maps = [
        {"x": xg[i * PLANES : (i + 1) * PLANES]} for i in range(N_CORES)
    ]
    res = run_bass_kernel_spmd(nc, in_maps, core_ids=list(range(N_CORES)))
    out = np.concatenate(
        [
            np.asarray(r["out"]).astype(np.float32).reshape(per_core, C, HO, WO)
            for r in res.results
        ],
        axis=0,
    )
    # v rows were stored at 2x (even cols) / 4x (odd cols) scale.
    out[:, :, 1::2, 0::2] *= 0.5
    out[:, :, 1::2, 1::2] *= 0.25
    return out



# revision 12
# speedup vs baseline: 2.0058x; 1.0215x over previous
"""Bilinear 2x upsample (16,3,512,512)->(16,3,1024,1024) on 8 trn2 NeuronCores.

Exact 2x bilinear: src = dst * 0.5, so
  out[2r, 2c]     = x[r, c]
  out[2r, 2c+1]   = 0.5*x[r, c]   + 0.5*x[r, c+1]   (clamped at c=511)
  out[2r+1, *]    = 0.5*row(2r,*) + 0.5*row(2r+2,*) (clamped at r=511)
All scale factors are powers of two, so the kernel reproduces the
reference bit-exactly (mul by 0.5 is exact; the adds round identically).

Sharding: pure data parallel, 2 images (= 6 512x512 planes) per core.

Per-core layout: each plane is loaded as T[128, 4, 512] with input row
r = 4p + b (partition p, free-dim block b). Horizontal interpolation
produces interleaved rows H[128, 4, 1024]; vertical averaging between
consecutive rows happens inside a partition (free-dim block shift) for
3/4 of the rows, and via a one-partition-shift SBUF->SBUF DMA of the
halved rows for the remaining block boundary (row 4p+3 pairs with row
4(p+1) which lives one partition down).
"""

import sys

if "/opt/trn_rl_repo" not in sys.path:
    sys.path.insert(0, "/opt/trn_rl_repo")

import numpy as np

N_CORES = 8
N, C, HI, WI = 16, 3, 512, 512
HO, WO = 1024, 1024
PLANES = (N // N_CORES) * C  # 6 planes per core
P = 128
B = HI // P  # 4 row-blocks per partition

_cached = {}


def _split_excess_waits(nc, max_waits=1):
    """Hoist excess sem waits into no-ops so each instruction carries <=max_waits.

    The walrus build in this container rejects instructions carrying more
    sync-wait commands than the ISA encoding slot count ("Too many sync wait
    commands", e.g. TPB_CTRL holds 1). Tile's scheduler attaches one wait per
    producer proc to a single instruction through an unchecked path. Waiting on
    a chain of same-engine no-ops immediately before the instruction is
    semantically identical (the engine stream is sequential), so move the
    excess waits there.
    """
    import concourse.mybir as mybir

    for f in nc.m.functions:
        for bb in f.blocks:
            insts = bb.instructions
            if not any(
                i.sync_info is not None and len(i.sync_info.on_wait) > max_waits
                for i in insts
            ):
                continue
            new = []
            for inst in insts:
                si = inst.sync_info
                if si is not None and len(si.on_wait) > max_waits:
                    waits = list(si.on_wait)
                    for w in waits[max_waits:]:
                        nop = mybir.InstNoOp(
                            name=nc.get_next_instruction_name(),
                            engine=inst.engine,
                            sync_info=mybir.SyncInfo(on_wait=[w], on_update=[]),
                            bass_nofuse=True,
                        )
                        nc.register_instruction(nop, overwrite=True)
                        new.append(nop)
                    inst.sync_info = mybir.SyncInfo(
                        on_wait=waits[:max_waits], on_update=list(si.on_update)
                    )
                new.append(inst)
            bb.instructions = new


def _build_module(reps=1, bufs=3):
    import concourse.bass as bass
    import concourse.mybir as mybir
    import concourse.tile as tile

    f32 = mybir.dt.float32
    bf16 = mybir.dt.bfloat16
    nc = bass.Bass()
    # x is the host-pre-gathered tile layout: [plane, partition, 5*512]
    # with x[pl, p, b*512 + w] = image[pl, min(4p+b, 511), w].
    x = nc.dram_tensor("x", [PLANES, P, (B + 1) * WI], f32, kind="ExternalInput")
    # Output is stored bf16 (rel err <= 2^-9, far under the 2e-2 gate) and
    # upcast to fp32 on the host: halves the dominant store traffic.
    out = nc.dram_tensor("out", [PLANES, HO, WO], bf16, kind="ExternalOutput")

    B5 = B + 1  # 4 owned row-blocks + 1 overlap block (row 4p+4)
    with tile.TileContext(nc) as tc:
        with (
            tc.tile_pool(name="tpool", bufs=PLANES) as tpool,
            tc.tile_pool(name="pool", bufs=bufs) as pool,
        ):
            # Per-plane combined fp32 tile T[p, k, b, w]: k=0 holds the
            # loaded input rows t (pre-gathered overlap layout, see _prep),
            # k=1 holds hso[b, j] = t_j + t_{j+1} (= 2 * h(row, 2j+1)).
            # Keeping t and hso in one tile lets ONE DVE tensor_add produce
            # both v-row column parities (see below).
            #
            # All loads are issued up front on the Sync FIFO, before any
            # store, so no load ever sits behind a store's compute-wait
            # (head-of-line blocking). Plane 0's load is split so the first
            # compute only waits for its first 3 row-blocks.
            Ts = []
            for pl in range(PLANES):
                T = tpool.tile([P, 2, B5, WI], f32)
                src = x[:][pl].rearrange("p (b w) -> p b w", b=B5)
                if pl == 0:
                    nc.sync.dma_start(T[:, 0, 0:3], src[:, 0:3])
                    nc.sync.dma_start(T[:, 0, 3:B5], src[:, 3:B5])
                else:
                    nc.sync.dma_start(T[:, 0], src)
                Ts.append(T)

            # All fp32 intermediates are bit-exact power-of-two multiples of
            # the reference values (power-of-two scaling commutes with fp32
            # rounding; each interpolation sum rounds exactly once, same as
            # the reference). Each stored element takes exactly ONE bf16
            # rounding, so |out - ref| <= |ref| * 2^-9 even where averages
            # cancel to ~0. v rows are stored at 2x (even cols) / 4x (odd
            # cols) scale — DVE adds can't scale their output — and fixed up
            # by exact power-of-two divides on the host.
            for pl in [p for _ in range(reps) for p in range(PLANES)]:
                T = Ts[pl]
                t, hso = T[:, 0], T[:, 1]
                hv = pool.tile([P, B, 2, WO], bf16)
                dst = out[:][pl].rearrange("(p b e) w -> p b e w", b=B, e=2)
                # First and last plane are processed in two half-plane
                # chunks: shortens the wait for the first store and the
                # tail after the last compute.
                chunks = ((0, 2), (2, 2)) if pl in (0, PLANES - 1) else ((0, 4),)
                for b0, nb in chunks:
                    hs0, hs1 = (b0, b0 + nb + 1) if b0 == 0 else (b0 + 1, B5)
                    nc.vector.tensor_add(
                        hso[:, hs0:hs1, 0 : WI - 1],
                        t[:, hs0:hs1, 0 : WI - 1],
                        t[:, hs0:hs1, 1:WI],
                    )
                    nc.scalar.mul(
                        hso[:, hs0:hs1, WI - 1 : WI],
                        t[:, hs0:hs1, WI - 1 : WI],
                        2.0,
                    )
                    # h rows (even out rows): even cols = t, odd = hso/2
                    nc.scalar.mul(hv[:, b0 : b0 + nb, 0, 0:WO:2], t[:, b0 : b0 + nb], 1.0)
                    nc.scalar.mul(hv[:, b0 : b0 + nb, 0, 1:WO:2], hso[:, b0 : b0 + nb], 0.5)
                    # v rows, both parities in ONE add: for k=0 (even cols,
                    # from t) and k=1 (odd cols, from hso),
                    # hv[b, 1, 2j+k] = T[k, b, j] + T[k, b+1, j].
                    nc.vector.tensor_add(
                        hv[:, b0 : b0 + nb, 1, :].rearrange("p b (j k) -> p k b j", k=2),
                        T[:, :, b0 : b0 + nb, :],
                        T[:, :, b0 + 1 : b0 + nb + 1, :],
                    )
                    nc.sync.dma_start(dst[:, b0 : b0 + nb], hv[:, b0 : b0 + nb])

    _split_excess_waits(nc)
    nc.finalize()
    return nc


def _get_module():
    if "nc" not in _cached:
        _cached["nc"] = _build_module()
    return _cached["nc"]


_ROW_IDX = np.minimum(
    4 * np.arange(P)[:, None] + np.arange(B + 1)[None, :], HI - 1
)  # [128, 5] source row per (partition, block)


def _prep(planes):
    """[n_planes, 512, 512] image planes -> [n_planes, 128, 2560] tile layout."""
    g = planes[:, _ROW_IDX, :]  # [n, 128, 5, 512]
    return np.ascontiguousarray(g.reshape(planes.shape[0], P, (B + 1) * WI))


def kernel(x, target_height=1024, target_width=1024):
    from concourse.bass_utils import run_bass_kernel_spmd

    assert int(target_height) == HO and int(target_width) == WO
    x = np.asarray(x, dtype=np.float32)
    assert x.shape == (N, C, HI, WI)
    xg = _prep(x.reshape(N * C, HI, WI))  # [48, 128, 2560]

    nc = _get_module()
    per_core = N // N_CORES
    in_maps = [
        {"x": xg[i * PLANES : (i + 1) * PLANES]} for i in range(N_CORES)
    ]
    res = run_bass_kernel_spmd(nc, in_maps, core_ids=list(range(N_CORES)))
    out = np.concatenate(
        [
            np.asarray(r["out"]).astype(np.float32).reshape(per_core, C, HO, WO)
            for r in res.results
        ],
        axis=0,
    )
    # v rows were stored at 2x (even cols) / 4x (odd cols) scale.
    out[:, :, 1::2, 0::2] *= 0.5
    out[:, :, 1::2, 1::2] *= 0.25
    return out



# revision 14
# speedup vs baseline: 2.0897x; 1.0418x over previous
"""Bilinear 2x upsample (16,3,512,512)->(16,3,1024,1024) on 8 trn2 NeuronCores.

Exact 2x bilinear: src = dst * 0.5, so
  out[2r, 2c]     = x[r, c]
  out[2r, 2c+1]   = 0.5*x[r, c]   + 0.5*x[r, c+1]   (clamped at c=511)
  out[2r+1, *]    = 0.5*row(2r,*) + 0.5*row(2r+2,*) (clamped at r=511)
All scale factors are powers of two, so the kernel reproduces the
reference bit-exactly (mul by 0.5 is exact; the adds round identically).

Sharding: pure data parallel, 2 images (= 6 512x512 planes) per core.

Per-core layout: each plane is loaded as T[128, 4, 512] with input row
r = 4p + b (partition p, free-dim block b). Horizontal interpolation
produces interleaved rows H[128, 4, 1024]; vertical averaging between
consecutive rows happens inside a partition (free-dim block shift) for
3/4 of the rows, and via a one-partition-shift SBUF->SBUF DMA of the
halved rows for the remaining block boundary (row 4p+3 pairs with row
4(p+1) which lives one partition down).
"""

import sys

if "/opt/trn_rl_repo" not in sys.path:
    sys.path.insert(0, "/opt/trn_rl_repo")

import numpy as np

N_CORES = 8
N, C, HI, WI = 16, 3, 512, 512
HO, WO = 1024, 1024
PLANES = (N // N_CORES) * C  # 6 planes per core
P = 128
B = HI // P  # 4 row-blocks per partition

_cached = {}


def _split_excess_waits(nc, max_waits=1):
    """Hoist excess sem waits into no-ops so each instruction carries <=max_waits.

    The walrus build in this container rejects instructions carrying more
    sync-wait commands than the ISA encoding slot count ("Too many sync wait
    commands", e.g. TPB_CTRL holds 1). Tile's scheduler attaches one wait per
    producer proc to a single instruction through an unchecked path. Waiting on
    a chain of same-engine no-ops immediately before the instruction is
    semantically identical (the engine stream is sequential), so move the
    excess waits there.
    """
    import concourse.mybir as mybir

    for f in nc.m.functions:
        for bb in f.blocks:
            insts = bb.instructions
            if not any(
                i.sync_info is not None and len(i.sync_info.on_wait) > max_waits
                for i in insts
            ):
                continue
            new = []
            for inst in insts:
                si = inst.sync_info
                if si is not None and len(si.on_wait) > max_waits:
                    waits = list(si.on_wait)
                    for w in waits[max_waits:]:
                        nop = mybir.InstNoOp(
                            name=nc.get_next_instruction_name(),
                            engine=inst.engine,
                            sync_info=mybir.SyncInfo(on_wait=[w], on_update=[]),
                            bass_nofuse=True,
                        )
                        nc.register_instruction(nop, overwrite=True)
                        new.append(nop)
                    inst.sync_info = mybir.SyncInfo(
                        on_wait=waits[:max_waits], on_update=list(si.on_update)
                    )
                new.append(inst)
            bb.instructions = new


def _build_module(reps=1, bufs=4):
    import concourse.bass as bass
    import concourse.mybir as mybir
    import concourse.tile as tile

    f32 = mybir.dt.float32
    bf16 = mybir.dt.bfloat16
    nc = bass.Bass()
    # x is the host-pre-gathered tile layout: [plane, partition, 5*512]
    # with x[pl, p, b*512 + w] = image[pl, min(4p+b, 511), w].
    x = nc.dram_tensor("x", [PLANES, P, (B + 1) * WI], f32, kind="ExternalInput")
    # Output is stored bf16 (rel err <= 2^-9, far under the 2e-2 gate) and
    # upcast to fp32 on the host: halves the dominant store traffic.
    out = nc.dram_tensor("out", [PLANES, HO, WO], bf16, kind="ExternalOutput")

    B5 = B + 1  # 4 owned row-blocks + 1 overlap block (row 4p+4)
    with tile.TileContext(nc) as tc:
        with (
            tc.tile_pool(name="tpool", bufs=PLANES) as tpool,
            tc.tile_pool(name="pool", bufs=bufs) as pool,
        ):
            # Per-plane combined fp32 tile T[p, k, b, w]: k=0 holds the
            # loaded input rows t (pre-gathered overlap layout, see _prep),
            # k=1 holds hso[b, j] = t_j + t_{j+1} (= 2 * h(row, 2j+1)).
            # Keeping t and hso in one tile lets ONE DVE tensor_add produce
            # both v-row column parities (see below).
            #
            # All loads are issued up front on the Sync FIFO, before any
            # store, so no load ever sits behind a store's compute-wait
            # (head-of-line blocking). Plane 0's load is split so the first
            # compute only waits for its first 3 row-blocks.
            Ts = []
            for pl in range(PLANES):
                T = tpool.tile([P, 2, B5, WI], f32)
                src = x[:][pl].rearrange("p (b w) -> p b w", b=B5)
                if pl == 0:
                    nc.sync.dma_start(T[:, 0, 0:3], src[:, 0:3])
                    nc.sync.dma_start(T[:, 0, 3:B5], src[:, 3:B5])
                else:
                    nc.sync.dma_start(T[:, 0], src)
                Ts.append(T)

            # All fp32 intermediates are bit-exact power-of-two multiples of
            # the reference values (power-of-two scaling commutes with fp32
            # rounding; each interpolation sum rounds exactly once, same as
            # the reference). Each stored element takes exactly ONE bf16
            # rounding, so |out - ref| <= |ref| * 2^-9 even where averages
            # cancel to ~0. v rows are stored at 2x (even cols) / 4x (odd
            # cols) scale — DVE adds can't scale their output — and fixed up
            # by exact power-of-two divides on the host.
            for pl in [p for _ in range(reps) for p in range(PLANES)]:
                T = Ts[pl]
                t, hso = T[:, 0], T[:, 1]
                hv = pool.tile([P, B, 2, WO], bf16)
                dst = out[:][pl].rearrange("(p b e) w -> p b e w", b=B, e=2)
                # First plane is processed in two half-plane chunks
                # (shortens the wait for the first store); the last two
                # planes in progressively finer chunks (shrinks the store
                # tail after the last compute).
                if pl == 0 or pl == PLANES - 2:
                    chunks = ((0, 2), (2, 2))
                elif pl == PLANES - 1:
                    chunks = ((0, 1), (1, 1), (2, 1), (3, 1))
                else:
                    chunks = ((0, 4),)
                hso_done = 0  # hso blocks [0, hso_done) already computed
                for b0, nb in chunks:
                    # this chunk's v rows need hso blocks [b0, b0+nb]
                    hs0, hs1 = hso_done, b0 + nb + 1
                    hso_done = hs1
                    nc.vector.tensor_add(
                        hso[:, hs0:hs1, 0 : WI - 1],
                        t[:, hs0:hs1, 0 : WI - 1],
                        t[:, hs0:hs1, 1:WI],
                    )
                    nc.scalar.mul(
                        hso[:, hs0:hs1, WI - 1 : WI],
                        t[:, hs0:hs1, WI - 1 : WI],
                        2.0,
                    )
                    # h rows (even out rows): even cols = t, odd = hso/2
                    nc.scalar.mul(hv[:, b0 : b0 + nb, 0, 0:WO:2], t[:, b0 : b0 + nb], 1.0)
                    nc.scalar.mul(hv[:, b0 : b0 + nb, 0, 1:WO:2], hso[:, b0 : b0 + nb], 0.5)
                    # v rows, both parities in ONE add: for k=0 (even cols,
                    # from t) and k=1 (odd cols, from hso),
                    # hv[b, 1, 2j+k] = T[k, b, j] + T[k, b+1, j].
                    nc.vector.tensor_add(
                        hv[:, b0 : b0 + nb, 1, :].rearrange("p b (j k) -> p k b j", k=2),
                        T[:, :, b0 : b0 + nb, :],
                        T[:, :, b0 + 1 : b0 + nb + 1, :],
                    )
                    nc.sync.dma_start(dst[:, b0 : b0 + nb], hv[:, b0 : b0 + nb])

    _split_excess_waits(nc)
    nc.finalize()
    return nc


def _get_module():
    if "nc" not in _cached:
        _cached["nc"] = _build_module()
    return _cached["nc"]


_ROW_IDX = np.minimum(
    4 * np.arange(P)[:, None] + np.arange(B + 1)[None, :], HI - 1
)  # [128, 5] source row per (partition, block)


def _prep(planes):
    """[n_planes, 512, 512] image planes -> [n_planes, 128, 2560] tile layout."""
    g = planes[:, _ROW_IDX, :]  # [n, 128, 5, 512]
    return np.ascontiguousarray(g.reshape(planes.shape[0], P, (B + 1) * WI))


def kernel(x, target_height=1024, target_width=1024):
    from concourse.bass_utils import run_bass_kernel_spmd

    assert int(target_height) == HO and int(target_width) == WO
    x = np.asarray(x, dtype=np.float32)
    assert x.shape == (N, C, HI, WI)
    xg = _prep(x.reshape(N * C, HI, WI))  # [48, 128, 2560]

    nc = _get_module()
    per_core = N // N_CORES
    in_maps = [
        {"x": xg[i * PLANES : (i + 1) * PLANES]} for i in range(N_CORES)
    ]
    res = run_bass_kernel_spmd(nc, in_maps, core_ids=list(range(N_CORES)))
    out = np.concatenate(
        [
            np.asarray(r["out"]).astype(np.float32).reshape(per_core, C, HO, WO)
            for r in res.results
        ],
        axis=0,
    )
    # v rows were stored at 2x (even cols) / 4x (odd cols) scale.
    out[:, :, 1::2, 0::2] *= 0.5
    out[:, :, 1::2, 1::2] *= 0.25
    return out



# revision 15
# speedup vs baseline: 2.0995x; 1.0047x over previous
"""Bilinear 2x upsample (16,3,512,512)->(16,3,1024,1024) on 8 trn2 NeuronCores.

Exact 2x bilinear: src = dst * 0.5, so
  out[2r, 2c]     = x[r, c]
  out[2r, 2c+1]   = 0.5*x[r, c]   + 0.5*x[r, c+1]   (clamped at c=511)
  out[2r+1, *]    = 0.5*row(2r,*) + 0.5*row(2r+2,*) (clamped at r=511)
All scale factors are powers of two, so the kernel reproduces the
reference bit-exactly (mul by 0.5 is exact; the adds round identically).

Sharding: pure data parallel, 2 images (= 6 512x512 planes) per core.

Per-core layout: each plane is loaded as T[128, 4, 512] with input row
r = 4p + b (partition p, free-dim block b). Horizontal interpolation
produces interleaved rows H[128, 4, 1024]; vertical averaging between
consecutive rows happens inside a partition (free-dim block shift) for
3/4 of the rows, and via a one-partition-shift SBUF->SBUF DMA of the
halved rows for the remaining block boundary (row 4p+3 pairs with row
4(p+1) which lives one partition down).
"""

import sys

if "/opt/trn_rl_repo" not in sys.path:
    sys.path.insert(0, "/opt/trn_rl_repo")

import numpy as np

N_CORES = 8
N, C, HI, WI = 16, 3, 512, 512
HO, WO = 1024, 1024
PLANES = (N // N_CORES) * C  # 6 planes per core
P = 128
B = HI // P  # 4 row-blocks per partition

_cached = {}


def _split_excess_waits(nc, max_waits=1):
    """Hoist excess sem waits into no-ops so each instruction carries <=max_waits.

    The walrus build in this container rejects instructions carrying more
    sync-wait commands than the ISA encoding slot count ("Too many sync wait
    commands", e.g. TPB_CTRL holds 1). Tile's scheduler attaches one wait per
    producer proc to a single instruction through an unchecked path. Waiting on
    a chain of same-engine no-ops immediately before the instruction is
    semantically identical (the engine stream is sequential), so move the
    excess waits there.
    """
    import concourse.mybir as mybir

    for f in nc.m.functions:
        for bb in f.blocks:
            insts = bb.instructions
            if not any(
                i.sync_info is not None and len(i.sync_info.on_wait) > max_waits
                for i in insts
            ):
                continue
            new = []
            for inst in insts:
                si = inst.sync_info
                if si is not None and len(si.on_wait) > max_waits:
                    waits = list(si.on_wait)
                    for w in waits[max_waits:]:
                        nop = mybir.InstNoOp(
                            name=nc.get_next_instruction_name(),
                            engine=inst.engine,
                            sync_info=mybir.SyncInfo(on_wait=[w], on_update=[]),
                            bass_nofuse=True,
                        )
                        nc.register_instruction(nop, overwrite=True)
                        new.append(nop)
                    inst.sync_info = mybir.SyncInfo(
                        on_wait=waits[:max_waits], on_update=list(si.on_update)
                    )
                new.append(inst)
            bb.instructions = new


def _build_module(reps=1, bufs=4):
    import concourse.bass as bass
    import concourse.mybir as mybir
    import concourse.tile as tile

    f32 = mybir.dt.float32
    bf16 = mybir.dt.bfloat16
    nc = bass.Bass()
    # x is the host-pre-gathered tile layout: [plane, partition, 5*512]
    # with x[pl, p, b*512 + w] = image[pl, min(4p+b, 511), w].
    x = nc.dram_tensor("x", [PLANES, P, (B + 1) * WI], f32, kind="ExternalInput")
    # Output is stored bf16 (rel err <= 2^-9, far under the 2e-2 gate) and
    # upcast to fp32 on the host: halves the dominant store traffic.
    out = nc.dram_tensor("out", [PLANES, HO, WO], bf16, kind="ExternalOutput")

    B5 = B + 1  # 4 owned row-blocks + 1 overlap block (row 4p+4)
    with tile.TileContext(nc) as tc:
        with (
            tc.tile_pool(name="tpool", bufs=PLANES) as tpool,
            tc.tile_pool(name="pool", bufs=bufs) as pool,
        ):
            # Per-plane combined fp32 tile T[p, k, b, w]: k=0 holds the
            # loaded input rows t (pre-gathered overlap layout, see _prep),
            # k=1 holds hso[b, j] = t_j + t_{j+1} (= 2 * h(row, 2j+1)).
            # Keeping t and hso in one tile lets ONE DVE tensor_add produce
            # both v-row column parities (see below).
            #
            # All loads are issued up front on the Sync FIFO, before any
            # store, so no load ever sits behind a store's compute-wait
            # (head-of-line blocking). Plane 0's load is split so the first
            # compute only waits for its first 3 row-blocks.
            Ts = []
            for pl in range(PLANES):
                T = tpool.tile([P, 2, B5, WI], f32)
                src = x[:][pl].rearrange("p (b w) -> p b w", b=B5)
                if pl == 0:
                    nc.sync.dma_start(T[:, 0, 0:2], src[:, 0:2])
                    nc.sync.dma_start(T[:, 0, 2:B5], src[:, 2:B5])
                else:
                    nc.sync.dma_start(T[:, 0], src)
                Ts.append(T)

            # All fp32 intermediates are bit-exact power-of-two multiples of
            # the reference values (power-of-two scaling commutes with fp32
            # rounding; each interpolation sum rounds exactly once, same as
            # the reference). Each stored element takes exactly ONE bf16
            # rounding, so |out - ref| <= |ref| * 2^-9 even where averages
            # cancel to ~0. v rows are stored at 2x (even cols) / 4x (odd
            # cols) scale — DVE adds can't scale their output — and fixed up
            # by exact power-of-two divides on the host.
            for pl in [p for _ in range(reps) for p in range(PLANES)]:
                T = Ts[pl]
                t, hso = T[:, 0], T[:, 1]
                hv = pool.tile([P, B, 2, WO], bf16)
                dst = out[:][pl].rearrange("(p b e) w -> p b e w", b=B, e=2)
                # First plane is processed in two half-plane chunks
                # (shortens the wait for the first store); the last two
                # planes in progressively finer chunks (shrinks the store
                # tail after the last compute).
                if pl == 0:
                    chunks = ((0, 1), (1, 3))
                elif pl == PLANES - 2:
                    chunks = ((0, 2), (2, 2))
                elif pl == PLANES - 1:
                    chunks = ((0, 1), (1, 1), (2, 1), (3, 1))
                else:
                    chunks = ((0, 4),)
                hso_done = 0  # hso blocks [0, hso_done) already computed
                for b0, nb in chunks:
                    # this chunk's v rows need hso blocks [b0, b0+nb]
                    hs0, hs1 = hso_done, b0 + nb + 1
                    hso_done = hs1
                    nc.vector.tensor_add(
                        hso[:, hs0:hs1, 0 : WI - 1],
                        t[:, hs0:hs1, 0 : WI - 1],
                        t[:, hs0:hs1, 1:WI],
                    )
                    nc.scalar.mul(
                        hso[:, hs0:hs1, WI - 1 : WI],
                        t[:, hs0:hs1, WI - 1 : WI],
                        2.0,
                    )
                    # h rows (even out rows): even cols = t, odd = hso/2
                    nc.scalar.mul(hv[:, b0 : b0 + nb, 0, 0:WO:2], t[:, b0 : b0 + nb], 1.0)
                    nc.scalar.mul(hv[:, b0 : b0 + nb, 0, 1:WO:2], hso[:, b0 : b0 + nb], 0.5)
                    # v rows, both parities in ONE add: for k=0 (even cols,
                    # from t) and k=1 (odd cols, from hso),
                    # hv[b, 1, 2j+k] = T[k, b, j] + T[k, b+1, j].
                    nc.vector.tensor_add(
                        hv[:, b0 : b0 + nb, 1, :].rearrange("p b (j k) -> p k b j", k=2),
                        T[:, :, b0 : b0 + nb, :],
                        T[:, :, b0 + 1 : b0 + nb + 1, :],
                    )
                    nc.sync.dma_start(dst[:, b0 : b0 + nb], hv[:, b0 : b0 + nb])

    _split_excess_waits(nc)
    nc.finalize()
    return nc


def _get_module():
    if "nc" not in _cached:
        _cached["nc"] = _build_module()
    return _cached["nc"]


_ROW_IDX = np.minimum(
    4 * np.arange(P)[:, None] + np.arange(B + 1)[None, :], HI - 1
)  # [128, 5] source row per (partition, block)


def _prep(planes):
    """[n_planes, 512, 512] image planes -> [n_planes, 128, 2560] tile layout."""
    g = planes[:, _ROW_IDX, :]  # [n, 128, 5, 512]
    return np.ascontiguousarray(g.reshape(planes.shape[0], P, (B + 1) * WI))


def kernel(x, target_height=1024, target_width=1024):
    from concourse.bass_utils import run_bass_kernel_spmd

    assert int(target_height) == HO and int(target_width) == WO
    x = np.asarray(x, dtype=np.float32)
    assert x.shape == (N, C, HI, WI)
    xg = _prep(x.reshape(N * C, HI, WI))  # [48, 128, 2560]

    nc = _get_module()
    per_core = N // N_CORES
    in_maps = [
        {"x": xg[i * PLANES : (i + 1) * PLANES]} for i in range(N_CORES)
    ]
    res = run_bass_kernel_spmd(nc, in_maps, core_ids=list(range(N_CORES)))
    out = np.concatenate(
        [
            np.asarray(r["out"]).astype(np.float32).reshape(per_core, C, HO, WO)
            for r in res.results
        ],
        axis=0,
    )
    # v rows were stored at 2x (even cols) / 4x (odd cols) scale.
    out[:, :, 1::2, 0::2] *= 0.5
    out[:, :, 1::2, 1::2] *= 0.25
    return out

